# revision 22
# baseline (speedup 1.0000x reference)
"""GRU-D cell on 8 NeuronCores via a Bass/Tile kernel.

Data-parallel: batch 16384 -> 8 x 2048; the 512x512 weights are
replicated. All device compute runs in a transposed [feature, batch]
layout so the matmul contraction dim sits on SBUF partitions; the host
pre-casts to bf16 and pre-transposes the four big per-element tensors
(x, m, delta_t, h_prev) so every device DMA is a plain contiguous copy.

Per 512-column batch chunk on each core (software-pipelined across
chunks; ACT/DVE queues are FIFO so emission order is execution order):
  gx    = exp(-relu(gamma)*dt)                  (ACT, one merged op)
  hd    = gx*h                                  (DVE)
  xd    = x*(m + gx - m*gx)                     (DVE, 3 fused ops; general
                                                 mean-imputation fallback)
  z/r   pre-acts accumulate over 8 matmuls per u-tile into 1-bank PSUM
  tau   = tanh(pre/2)  [sigmoid via tanh: one ACT table set, no reloads]
  rh'   = (tau_r+1)*hd   with U_h host-scaled by 0.5
  h_hat = tanh(W_h xd + U_h' rh')
  h_new = hd + (0.5*tau_z+0.5)*(h_hat-hd)       (DVE fused)

Weights ride the GpSimd SWDGE queue so chunk loads on the Sync HWDGE
queue aren't serialized behind them. The program is built lazily per
specialization (uniform gamma decays / zero mean-imputation / zero
biases; general per-tile fallback otherwise).
"""

import numpy as np
import ml_dtypes

F = 512          # feature dim == units
B = 16384        # full batch
N_CORES = 8
BC = B // N_CORES     # per-core batch rows (2048)
NB = 512              # max batch-column chunk (matmul free dim)
# two small leading chunks: matmuls start while the startup DMA backlog
# drains (PE has slack there anyway); steady-state and tail stay 512-wide
CHUNK_W = (256, 256, 512, 512, 512)
CHUNK_OFF = (0, 256, 512, 1024, 1536)
KT = F // 128         # feature tiles (4)

BF16 = ml_dtypes.bfloat16
FP8 = ml_dtypes.float8_e4m3
# fp8e4m3 + DoubleRow for the x_decayed @ W_* matmuls (W-side) halves those
# matmul counts (~10us), but measured rel-err lands at 1.99e-2 vs the 2e-2
# gate -- no safety margin, so it stays off.
FP8_W = False

_STATE = {}


def _build(spec_key):
    """spec_key: ("spec", gx_scale) for the specialized program, or "gen"."""
    import concourse.mybir as mybir
    from concourse import bacc
    from concourse.tile import TileContext, add_dep_helper

    dt = mybir.dt
    AF = mybir.ActivationFunctionType
    OP = mybir.AluOpType

    specialized = spec_key[0] == "spec"
    w_dt = dt.float8e4 if FP8_W else dt.bfloat16

    nc = bacc.Bacc("TRN2", num_devices=N_CORES, debug=False)

    x_d = nc.dram_tensor("x", [F, BC], dt.bfloat16, kind="ExternalInput").ap()
    m_d = nc.dram_tensor("m", [F, BC], dt.bfloat16, kind="ExternalInput").ap()
    d_d = nc.dram_tensor("d", [F, BC], dt.bfloat16, kind="ExternalInput").ap()
    h_d = nc.dram_tensor("h", [F, BC], dt.bfloat16, kind="ExternalInput").ap()
    w_d = nc.dram_tensor("w", [F, 3 * F], w_dt, kind="ExternalInput").ap()
    u_d = nc.dram_tensor("u", [F, 3 * F], dt.bfloat16, kind="ExternalInput").ap()
    if not specialized:
        c_d = nc.dram_tensor("c", [128, KT, 6], dt.float32, kind="ExternalInput").ap()
    o_d = nc.dram_tensor("o", [F, BC], dt.bfloat16, kind="ExternalOutput").ap()

    NGX, NGH, MI, BZ, BR, BH = range(6)

    x_r = x_d.rearrange("(kt p) b -> p kt b", p=128)
    m_r = m_d.rearrange("(kt p) b -> p kt b", p=128)
    d_r = d_d.rearrange("(kt p) b -> p kt b", p=128)
    h_r = h_d.rearrange("(kt p) b -> p kt b", p=128)
    w_r = w_d.rearrange("(kt p) n -> p kt n", p=128)
    u_r = u_d.rearrange("(kt p) n -> p kt n", p=128)
    o_r = o_d.rearrange("(ut p) b -> p ut b", p=128)

    with TileContext(nc) as tc:
        with (
            tc.tile_pool(name="const", bufs=1) as cpool,
            tc.tile_pool(name="io", bufs=2) as io,
            tc.tile_pool(name="work", bufs=2) as wk,
            tc.tile_pool(name="tmp", bufs=3) as tp,
            tc.tile_pool(name="psum", bufs=8, space="PSUM") as pp,
        ):
            wsb = cpool.tile([128, KT, 3 * F], w_dt, tag="wsb")
            usb = cpool.tile([128, KT, 3 * F], dt.bfloat16, tag="usb")
            if not specialized:
                csb = cpool.tile([128, KT, 6], dt.float32, tag="csb")
                nc.sync.dma_start(out=csb[:], in_=c_d)

            def load_chunk_dh(c, after=None):
                w = CHUNK_W[c]
                rows = slice(CHUNK_OFF[c], CHUNK_OFF[c] + w)
                dT = io.tile([128, KT, w], dt.bfloat16, tag="dT")
                hT = io.tile([128, KT, w], dt.bfloat16, tag="hT")
                i1 = nc.sync.dma_start(out=dT[:], in_=d_r[:, :, rows])
                i2 = nc.sync.dma_start(out=hT[:], in_=h_r[:, :, rows])
                for i in (i1, i2):
                    if after is not None:
                        add_dep_helper(i.ins, after.ins, reason="stage DMA waves")
                return dT, hT, i2

            def load_chunk_xm(c, after=None):
                w = CHUNK_W[c]
                rows = slice(CHUNK_OFF[c], CHUNK_OFF[c] + w)
                xT = io.tile([128, KT, w], dt.bfloat16, tag="xT")
                mT = io.tile([128, KT, w], dt.bfloat16, tag="mT")
                i1 = nc.sync.dma_start(out=xT[:], in_=x_r[:, :, rows])
                i2 = nc.sync.dma_start(out=mT[:], in_=m_r[:, :, rows])
                for i in (i1, i2):
                    if after is not None:
                        add_dep_helper(i.ins, after.ins, reason="stage DMA waves")
                return xT, mT, i2

            def exp_chunk(tiles):
                dT, hT, xT, mT = tiles
                w = dT.shape[2]
                if specialized:
                    gx = tp.tile([128, KT, w], dt.bfloat16, tag="gx")
                    nc.scalar.activation(gx[:], dT[:], AF.Exp, scale=spec_key[1])
                    return (gx,)
                gxs, ghs = [], []
                w = dT.shape[2]
                for t in range(KT):
                    gh = tp.tile([128, w], dt.bfloat16, tag=f"gh{t}")
                    nc.scalar.activation(gh[:], dT[:, t, :], AF.Exp,
                                         scale=csb[:, t, NGH:NGH + 1])
                    gxt = tp.tile([128, w], dt.bfloat16, tag=f"gxt{t}")
                    nc.scalar.activation(gxt[:], dT[:, t, :], AF.Exp,
                                         scale=csb[:, t, NGX:NGX + 1])
                    gxs.append(gxt); ghs.append(gh)
                return (gxs, ghs)

            def preproc_chunk(tiles, gtiles):
                dT, hT, xT, mT = tiles
                w = dT.shape[2]
                hd = wk.tile([128, KT, w], dt.bfloat16, tag="hd")
                xd = wk.tile([128, KT, w], dt.float8e4 if FP8_W else dt.bfloat16, tag="xd")
                if specialized:
                    (gx,) = gtiles
                    nc.vector.tensor_mul(out=hd[:], in0=gx[:], in1=hT[:])
                    # xd = x*(m + gx - m*gx): bm=(gx-1)*m ; s=gx-bm ; xd=x*s
                    bm = tp.tile([128, KT, w], dt.bfloat16, tag="bm")
                    nc.vector.scalar_tensor_tensor(
                        bm[:], gx[:], 1.0, mT[:], OP.subtract, OP.mult)
                    s = tp.tile([128, KT, w], dt.bfloat16, tag="s")
                    nc.vector.tensor_sub(out=s[:], in0=gx[:], in1=bm[:])
                    nc.vector.tensor_mul(out=xd[:], in0=xT[:], in1=s[:])
                else:
                    gxs, ghs = gtiles
                    for t in range(KT):
                        nc.vector.tensor_mul(out=hd[:, t, :], in0=ghs[t][:],
                                             in1=hT[:, t, :])
                        # v = gx*(x-mi)+mi ; xd = v + m*(x-v)
                        p1 = tp.tile([128, w], dt.bfloat16, tag="p1")
                        nc.vector.scalar_tensor_tensor(
                            p1[:], xT[:, t, :], csb[:, t, MI:MI + 1], gxs[t][:],
                            OP.subtract, OP.mult)
                        v = tp.tile([128, w], dt.bfloat16, tag="v")
                        nc.vector.tensor_scalar(v[:], p1[:], csb[:, t, MI:MI + 1],
                                                None, OP.add)
                        q = tp.tile([128, w], dt.bfloat16, tag="q")
                        nc.vector.tensor_sub(out=q[:], in0=xT[:, t, :], in1=v[:])
                        r2 = tp.tile([128, w], dt.bfloat16, tag="r2")
                        nc.vector.tensor_mul(out=r2[:], in0=mT[:, t, :], in1=q[:])
                        nc.vector.tensor_add(out=xd[:, t, :], in0=v[:], in1=r2[:])
                return hd, xd

            def side_mms(ps, base, ut, lhs, rhs, start, stop):
                colw = slice(base + ut * 128, base + (ut + 1) * 128)
                if lhs.dtype == dt.float8e4:
                    # fp8 DoubleRow: one MM per k-subtile pair
                    for i in range(KT // 2):
                        nc.tensor.matmul(
                            ps[:], lhs[:, 2 * i:2 * i + 2, colw],
                            rhs[:, 2 * i:2 * i + 2, :],
                            start=(start and i == 0),
                            stop=(stop and i == KT // 2 - 1),
                            perf_mode=mybir.MatmulPerfMode.DoubleRow)
                else:
                    for t in range(KT):
                        nc.tensor.matmul(ps[:], lhs[:, t, colw], rhs[:, t, :],
                                         start=(start and t == 0),
                                         stop=(stop and t == KT - 1))

            def gate_mms(base, lhs_a, rhs_a, lhs_b, rhs_b):
                """Accumulating MMs per u-tile into four 1-bank psums.
                All lhs_a MMs first so the lhs_b weights get extra DMA time
                at kernel start."""
                pss = []
                for ut in range(KT):
                    ps = pp.tile([128, rhs_a.shape[2]], dt.float32, tag="ps")
                    side_mms(ps, base, ut, lhs_a, rhs_a, True, False)
                    pss.append(ps)
                for ut in range(KT):
                    side_mms(pss[ut], base, ut, lhs_b, rhs_b, False, True)
                return pss

            def tau_acts(pss, boff, tag):
                tau = wk.tile([128, KT, pss[0].shape[1]], dt.bfloat16, tag=tag)
                for ut in range(KT):
                    if specialized:
                        nc.scalar.activation(tau[:, ut, :], pss[ut][:], AF.Tanh,
                                             scale=0.5)
                    else:
                        nc.scalar.activation(tau[:, ut, :], pss[ut][:], AF.Tanh,
                                             bias=csb[:, ut, boff:boff + 1],
                                             scale=0.5)
                return tau

            # ---- prologue: staged DMA waves. All outstanding DMAs share
            # HBM bandwidth concurrently, so gate non-critical transfers
            # behind the critical ones with explicit deps. The r gate runs
            # first (its tau feeds rh -> h matmuls), so U_r leads.
            # W1 (free): d0, h0 + U_r first half.
            dT0, hT0, i_h0 = load_chunk_dh(0)
            i_ur1 = nc.gpsimd.dma_start(out=usb[:, :, F:F + 256],
                                        in_=u_r[:, :, F:F + 256])
            # W2 (after h0): x0, m0, U_r second half, W_r.
            xT0, mT0, i_m0 = load_chunk_xm(0, after=i_h0)
            i_ur2 = nc.gpsimd.dma_start(out=usb[:, :, F + 256:2 * F],
                                        in_=u_r[:, :, F + 256:2 * F])
            i_wr = nc.gpsimd.dma_start(out=wsb[:, :, F:2 * F], in_=w_r[:, :, F:2 * F])
            for i in (i_ur2, i_wr):
                add_dep_helper(i.ins, i_h0.ins, reason="stage DMA waves")
            tiles = (dT0, hT0, xT0, mT0)
            gtiles = exp_chunk(tiles)
            hd, xd = preproc_chunk(tiles, gtiles)
            # W3 (after m0): remaining weights in first-use order (z then h).
            i_wlast = None
            for lo, hi, dst, src in (
                (0, F, usb, u_r), (0, F, wsb, w_r),
                (2 * F, 3 * F, wsb, w_r), (2 * F, 3 * F, usb, u_r),
            ):
                i_wlast = nc.gpsimd.dma_start(out=dst[:, :, lo:hi], in_=src[:, :, lo:hi])
                add_dep_helper(i_wlast.ins, i_m0.ins, reason="stage DMA waves")

            for c in range(len(CHUNK_W)):
                rows = slice(CHUNK_OFF[c], CHUNK_OFF[c] + CHUNK_W[c])
                last = c == len(CHUNK_W) - 1

                ps_r = gate_mms(F, usb, hd, wsb, xd)
                ps_z = gate_mms(0, usb, hd, wsb, xd)
                tau_r = tau_acts(ps_r, BR, "tau_r")
                tau_z = tau_acts(ps_z, BZ, "tau_z")

                if not last:
                    # chunk-1 loads wait for the last weight DMA so the
                    # startup waves aren't diluted; later chunks are free.
                    gate = i_wlast if c == 0 else None
                    dTn, hTn, i_hn = load_chunk_dh(c + 1, after=gate)
                    xTn, mTn, _ = load_chunk_xm(c + 1, after=gate)
                    tiles_n = (dTn, hTn, xTn, mTn)
                    gtiles_n = exp_chunk(tiles_n)

                # rh' = (tau_r + 1) * hd   (U_h is host-scaled by 0.5)
                rh = wk.tile([128, KT, CHUNK_W[c]], dt.bfloat16, tag="rh")
                nc.vector.scalar_tensor_tensor(
                    rh[:], tau_r[:], 1.0, hd[:], OP.add, OP.mult)

                ps_h = gate_mms(2 * F, wsb, xd, usb, rh)

                hd_c, xd_c = hd, xd
                if not last:
                    hd, xd = preproc_chunk(tiles_n, gtiles_n)
                    tiles = tiles_n

                # h_hat = tanh(.+b_h); h_new = hd + (0.5 tau_z + 0.5)(hh-hd)
                cw = CHUNK_W[c]
                if last:
                    # per-u-tile so the kernel tail pipelines
                    for ut in range(KT):
                        hh = tp.tile([128, cw], dt.bfloat16, tag="hh")
                        if specialized:
                            nc.scalar.activation(hh[:], ps_h[ut][:], AF.Tanh)
                        else:
                            nc.scalar.activation(hh[:], ps_h[ut][:], AF.Tanh,
                                                 bias=csb[:, ut, BH:BH + 1])
                        t6 = tp.tile([128, cw], dt.bfloat16, tag="t6")
                        nc.vector.tensor_sub(out=t6[:], in0=hh[:], in1=hd_c[:, ut, :])
                        t7 = tp.tile([128, cw], dt.bfloat16, tag="t7")
                        nc.vector.scalar_tensor_tensor(
                            t7[:], tau_z[:, ut, :], 1.0, t6[:], OP.add, OP.mult)
                        hn = tp.tile([128, cw], dt.bfloat16, tag="hn")
                        nc.vector.scalar_tensor_tensor(
                            hn[:], t7[:], 0.5, hd_c[:, ut, :], OP.mult, OP.add)
                        nc.sync.dma_start(out=o_r[:, ut, rows], in_=hn[:])
                else:
                    hh = wk.tile([128, KT, cw], dt.bfloat16, tag="hhm")
                    for ut in range(KT):
                        if specialized:
                            nc.scalar.activation(hh[:, ut, :], ps_h[ut][:], AF.Tanh)
                        else:
                            nc.scalar.activation(hh[:, ut, :], ps_h[ut][:], AF.Tanh,
                                                 bias=csb[:, ut, BH:BH + 1])
                    t6 = tp.tile([128, KT, cw], dt.bfloat16, tag="t6m")
                    nc.vector.tensor_sub(out=t6[:], in0=hh[:], in1=hd_c[:])
                    t7 = tp.tile([128, KT, cw], dt.bfloat16, tag="t7m")
                    nc.vector.scalar_tensor_tensor(
                        t7[:], tau_z[:], 1.0, t6[:], OP.add, OP.mult)
                    hn = wk.tile([128, KT, cw], dt.bfloat16, tag="hnm")
                    nc.vector.scalar_tensor_tensor(
                        hn[:], t7[:], 0.5, hd_c[:], OP.mult, OP.add)
                    nc.sync.dma_start(out=o_r[:, :, rows], in_=hn[:])

    nc.compile()
    return nc


def _get_nc(spec_key):
    if spec_key not in _STATE:
        _STATE[spec_key] = _build(spec_key)
    return _STATE[spec_key]


def _tp_cast(a):
    """[B, F] f32 view -> [F, B] contiguous bf16."""
    return np.ascontiguousarray(a.T).astype(BF16)


def kernel(**inputs) -> np.ndarray:
    from concourse import bass_utils

    inp = np.asarray(inputs["inputs"], dtype=np.float32)
    h_prev = np.asarray(inputs["h_prev"], dtype=np.float32)
    gx = np.maximum(np.asarray(inputs["gamma_x_decay"], np.float32), 0.0)
    gh = np.maximum(np.asarray(inputs["gamma_h_decay"], np.float32), 0.0)
    mi = np.asarray(inputs["mean_imputation"], np.float32)
    bz = np.asarray(inputs["b_z"], np.float32)
    br = np.asarray(inputs["b_r"], np.float32)
    bh = np.asarray(inputs["b_h"], np.float32)

    specialized = bool(
        np.all(gx == gx[0]) and np.all(gh == gx[0])
        and not np.any(mi) and not np.any(bz) and not np.any(br) and not np.any(bh)
    )
    spec_key = ("spec", float(-gx[0])) if specialized else "gen"
    nc = _get_nc(spec_key)

    xT = _tp_cast(inp[:, :F])
    mT = _tp_cast(inp[:, F:2 * F])
    dT = _tp_cast(inp[:, 2 * F:])
    hT = _tp_cast(h_prev)

    w = np.concatenate(
        [np.asarray(inputs["W_z"]), np.asarray(inputs["W_r"]), np.asarray(inputs["W_h"])],
        axis=1).astype(FP8 if FP8_W else BF16)
    # fold the sigmoid-via-tanh 0.5 rescale of r into U_h
    u = np.concatenate(
        [np.asarray(inputs["U_z"]), np.asarray(inputs["U_r"]),
         0.5 * np.asarray(inputs["U_h"])],
        axis=1).astype(BF16)

    in_maps = []
    for c in range(N_CORES):
        cols = slice(c * BC, (c + 1) * BC)
        im = {"x": xT[:, cols], "m": mT[:, cols], "d": dT[:, cols], "h": hT[:, cols],
              "w": w, "u": u}
        if not specialized:
            # half-biases for z/r: tanh((pre+b)/2) takes b/2 as the ACT bias
            consts = np.stack([-gx, -gh, mi, 0.5 * bz, 0.5 * br, bh], axis=-1)
            im["c"] = np.ascontiguousarray(
                consts.reshape(KT, 128, 6).transpose(1, 0, 2))
        in_maps.append(im)

    res = bass_utils.run_bass_kernel_spmd(
        nc, in_maps, core_ids=list(range(N_CORES)), **_STATE.get("run_kwargs", {})
    )
    _STATE["last_results"] = res

    out = np.empty((B, F), np.float32)
    for c in range(N_CORES):
        out[c * BC:(c + 1) * BC, :] = res.results[c]["o"].T.astype(np.float32)
    return out


# revision 24
# speedup vs baseline: 1.0659x; 1.0659x over previous
"""GRU-D cell on 8 NeuronCores via a Bass/Tile kernel.

Data-parallel: batch 16384 -> 8 x 2048; the 512x512 weights are
replicated. All device compute runs in a transposed [feature, batch]
layout so the matmul contraction dim sits on SBUF partitions; the host
pre-casts to bf16 and pre-transposes the four big per-element tensors
(x, m, delta_t, h_prev) so every device DMA is a plain contiguous copy.

Per 512-column batch chunk on each core (software-pipelined across
chunks; ACT/DVE queues are FIFO so emission order is execution order):
  gx    = exp(-relu(gamma)*dt)                  (ACT, one merged op)
  hd    = gx*h                                  (DVE)
  xd    = x*(m + gx - m*gx)                     (DVE, 3 fused ops; general
                                                 mean-imputation fallback)
  z/r   pre-acts accumulate over 8 matmuls per u-tile into 1-bank PSUM
  tau   = tanh(pre/2)  [sigmoid via tanh: one ACT table set, no reloads]
  rh'   = (tau_r+1)*hd   with U_h host-scaled by 0.5
  h_hat = tanh(W_h xd + U_h' rh')
  h_new = hd + (0.5*tau_z+0.5)*(h_hat-hd)       (DVE fused)

Weights ride the GpSimd SWDGE queue so chunk loads on the Sync HWDGE
queue aren't serialized behind them. The program is built lazily per
specialization (uniform gamma decays / zero mean-imputation / zero
biases; general per-tile fallback otherwise).
"""

import numpy as np
import ml_dtypes

F = 512          # feature dim == units
B = 16384        # full batch
N_CORES = 8
BC = B // N_CORES     # per-core batch rows (2048)
NB = 512              # max batch-column chunk (matmul free dim)
# uniform 512-wide chunks: narrower chunks start matmuls sooner but add
# chunk-boundary dependency stalls that cost more than they save (measured)
CHUNK_W = (512, 512, 512, 512)
CHUNK_OFF = (0, 512, 1024, 1536)
KT = F // 128         # feature tiles (4)

BF16 = ml_dtypes.bfloat16
FP8 = ml_dtypes.float8_e4m3
# fp8e4m3 + DoubleRow for the x_decayed @ W_* matmuls (W-side) halves those
# matmul counts (~10us), but measured rel-err lands at 1.99e-2 vs the 2e-2
# gate -- no safety margin, so it stays off.
FP8_W = False

_STATE = {}


def _build(spec_key):
    """spec_key: ("spec", gx_scale) for the specialized program, or "gen"."""
    import concourse.mybir as mybir
    from concourse import bacc
    from concourse.tile import TileContext, add_dep_helper

    dt = mybir.dt
    AF = mybir.ActivationFunctionType
    OP = mybir.AluOpType

    specialized = spec_key[0] == "spec"
    w_dt = dt.float8e4 if FP8_W else dt.bfloat16

    nc = bacc.Bacc("TRN2", num_devices=N_CORES, debug=False)

    x_d = nc.dram_tensor("x", [F, BC], dt.bfloat16, kind="ExternalInput").ap()
    m_d = nc.dram_tensor("m", [F, BC], dt.bfloat16, kind="ExternalInput").ap()
    d_d = nc.dram_tensor("d", [F, BC], dt.bfloat16, kind="ExternalInput").ap()
    h_d = nc.dram_tensor("h", [F, BC], dt.bfloat16, kind="ExternalInput").ap()
    w_d = nc.dram_tensor("w", [F, 3 * F], w_dt, kind="ExternalInput").ap()
    u_d = nc.dram_tensor("u", [F, 3 * F], dt.bfloat16, kind="ExternalInput").ap()
    if not specialized:
        c_d = nc.dram_tensor("c", [128, KT, 6], dt.float32, kind="ExternalInput").ap()
    o_d = nc.dram_tensor("o", [F, BC], dt.bfloat16, kind="ExternalOutput").ap()

    NGX, NGH, MI, BZ, BR, BH = range(6)

    x_r = x_d.rearrange("(kt p) b -> p kt b", p=128)
    m_r = m_d.rearrange("(kt p) b -> p kt b", p=128)
    d_r = d_d.rearrange("(kt p) b -> p kt b", p=128)
    h_r = h_d.rearrange("(kt p) b -> p kt b", p=128)
    w_r = w_d.rearrange("(kt p) n -> p kt n", p=128)
    u_r = u_d.rearrange("(kt p) n -> p kt n", p=128)
    o_r = o_d.rearrange("(ut p) b -> p ut b", p=128)

    with TileContext(nc) as tc:
        with (
            tc.tile_pool(name="const", bufs=1) as cpool,
            tc.tile_pool(name="io", bufs=2) as io,
            tc.tile_pool(name="work", bufs=2) as wk,
            tc.tile_pool(name="tmp", bufs=3) as tp,
            tc.tile_pool(name="psum", bufs=8, space="PSUM") as pp,
        ):
            wsb = cpool.tile([128, KT, 3 * F], w_dt, tag="wsb")
            usb = cpool.tile([128, KT, 3 * F], dt.bfloat16, tag="usb")
            if not specialized:
                csb = cpool.tile([128, KT, 6], dt.float32, tag="csb")
                nc.sync.dma_start(out=csb[:], in_=c_d)

            def load_chunk_dh(c, after=None):
                w = CHUNK_W[c]
                rows = slice(CHUNK_OFF[c], CHUNK_OFF[c] + w)
                dT = io.tile([128, KT, w], dt.bfloat16, tag="dT")
                hT = io.tile([128, KT, w], dt.bfloat16, tag="hT")
                i1 = nc.sync.dma_start(out=dT[:], in_=d_r[:, :, rows])
                i2 = nc.sync.dma_start(out=hT[:], in_=h_r[:, :, rows])
                for i in (i1, i2):
                    if after is not None:
                        add_dep_helper(i.ins, after.ins, reason="stage DMA waves")
                return dT, hT, i2

            def load_chunk_xm(c, after=None):
                w = CHUNK_W[c]
                rows = slice(CHUNK_OFF[c], CHUNK_OFF[c] + w)
                xT = io.tile([128, KT, w], dt.bfloat16, tag="xT")
                mT = io.tile([128, KT, w], dt.bfloat16, tag="mT")
                i1 = nc.sync.dma_start(out=xT[:], in_=x_r[:, :, rows])
                i2 = nc.sync.dma_start(out=mT[:], in_=m_r[:, :, rows])
                for i in (i1, i2):
                    if after is not None:
                        add_dep_helper(i.ins, after.ins, reason="stage DMA waves")
                return xT, mT, i2

            def exp_chunk(tiles):
                dT, hT, xT, mT = tiles
                w = dT.shape[2]
                if specialized:
                    gx = tp.tile([128, KT, w], dt.bfloat16, tag="gx")
                    nc.scalar.activation(gx[:], dT[:], AF.Exp, scale=spec_key[1])
                    return (gx,)
                gxs, ghs = [], []
                w = dT.shape[2]
                for t in range(KT):
                    gh = tp.tile([128, w], dt.bfloat16, tag=f"gh{t}")
                    nc.scalar.activation(gh[:], dT[:, t, :], AF.Exp,
                                         scale=csb[:, t, NGH:NGH + 1])
                    gxt = tp.tile([128, w], dt.bfloat16, tag=f"gxt{t}")
                    nc.scalar.activation(gxt[:], dT[:, t, :], AF.Exp,
                                         scale=csb[:, t, NGX:NGX + 1])
                    gxs.append(gxt); ghs.append(gh)
                return (gxs, ghs)

            def preproc_chunk(tiles, gtiles):
                dT, hT, xT, mT = tiles
                w = dT.shape[2]
                hd = wk.tile([128, KT, w], dt.bfloat16, tag="hd")
                xd = wk.tile([128, KT, w], dt.float8e4 if FP8_W else dt.bfloat16, tag="xd")
                if specialized:
                    (gx,) = gtiles
                    nc.vector.tensor_mul(out=hd[:], in0=gx[:], in1=hT[:])
                    # xd = x*(m + gx - m*gx): bm=(gx-1)*m ; s=gx-bm ; xd=x*s
                    bm = tp.tile([128, KT, w], dt.bfloat16, tag="bm")
                    nc.vector.scalar_tensor_tensor(
                        bm[:], gx[:], 1.0, mT[:], OP.subtract, OP.mult)
                    s = tp.tile([128, KT, w], dt.bfloat16, tag="s")
                    nc.vector.tensor_sub(out=s[:], in0=gx[:], in1=bm[:])
                    nc.vector.tensor_mul(out=xd[:], in0=xT[:], in1=s[:])
                else:
                    gxs, ghs = gtiles
                    for t in range(KT):
                        nc.vector.tensor_mul(out=hd[:, t, :], in0=ghs[t][:],
                                             in1=hT[:, t, :])
                        # v = gx*(x-mi)+mi ; xd = v + m*(x-v)
                        p1 = tp.tile([128, w], dt.bfloat16, tag="p1")
                        nc.vector.scalar_tensor_tensor(
                            p1[:], xT[:, t, :], csb[:, t, MI:MI + 1], gxs[t][:],
                            OP.subtract, OP.mult)
                        v = tp.tile([128, w], dt.bfloat16, tag="v")
                        nc.vector.tensor_scalar(v[:], p1[:], csb[:, t, MI:MI + 1],
                                                None, OP.add)
                        q = tp.tile([128, w], dt.bfloat16, tag="q")
                        nc.vector.tensor_sub(out=q[:], in0=xT[:, t, :], in1=v[:])
                        r2 = tp.tile([128, w], dt.bfloat16, tag="r2")
                        nc.vector.tensor_mul(out=r2[:], in0=mT[:, t, :], in1=q[:])
                        nc.vector.tensor_add(out=xd[:, t, :], in0=v[:], in1=r2[:])
                return hd, xd

            def side_mms(ps, base, ut, lhs, rhs, start, stop):
                colw = slice(base + ut * 128, base + (ut + 1) * 128)
                if lhs.dtype == dt.float8e4:
                    # fp8 DoubleRow: one MM per k-subtile pair
                    for i in range(KT // 2):
                        nc.tensor.matmul(
                            ps[:], lhs[:, 2 * i:2 * i + 2, colw],
                            rhs[:, 2 * i:2 * i + 2, :],
                            start=(start and i == 0),
                            stop=(stop and i == KT // 2 - 1),
                            perf_mode=mybir.MatmulPerfMode.DoubleRow)
                else:
                    for t in range(KT):
                        nc.tensor.matmul(ps[:], lhs[:, t, colw], rhs[:, t, :],
                                         start=(start and t == 0),
                                         stop=(stop and t == KT - 1))

            def gate_mms(base, lhs_a, rhs_a, lhs_b, rhs_b):
                """Accumulating MMs per u-tile into four 1-bank psums.
                All lhs_a MMs first so the lhs_b weights get extra DMA time
                at kernel start."""
                pss = []
                for ut in range(KT):
                    ps = pp.tile([128, rhs_a.shape[2]], dt.float32, tag="ps")
                    side_mms(ps, base, ut, lhs_a, rhs_a, True, False)
                    pss.append(ps)
                for ut in range(KT):
                    side_mms(pss[ut], base, ut, lhs_b, rhs_b, False, True)
                return pss

            def tau_acts(pss, boff, tag):
                tau = wk.tile([128, KT, pss[0].shape[1]], dt.bfloat16, tag=tag)
                for ut in range(KT):
                    if specialized:
                        nc.scalar.activation(tau[:, ut, :], pss[ut][:], AF.Tanh,
                                             scale=0.5)
                    else:
                        nc.scalar.activation(tau[:, ut, :], pss[ut][:], AF.Tanh,
                                             bias=csb[:, ut, boff:boff + 1],
                                             scale=0.5)
                return tau

            # ---- prologue: staged DMA waves. All outstanding DMAs share
            # HBM bandwidth concurrently, so gate non-critical transfers
            # behind the critical ones with explicit deps. The r gate runs
            # first (its tau feeds rh -> h matmuls), so U_r leads.
            # W1 (free): chunk-0 tensors + U_r.
            dT0, hT0, i_h0 = load_chunk_dh(0)
            nc.gpsimd.dma_start(out=usb[:, :, F:2 * F], in_=u_r[:, :, F:2 * F])
            xT0, mT0, i_m0 = load_chunk_xm(0)
            # W2 (after h0): W_r.
            i_wr = nc.gpsimd.dma_start(out=wsb[:, :, F:2 * F], in_=w_r[:, :, F:2 * F])
            add_dep_helper(i_wr.ins, i_h0.ins, reason="stage DMA waves")
            tiles = (dT0, hT0, xT0, mT0)
            gtiles = exp_chunk(tiles)
            hd, xd = preproc_chunk(tiles, gtiles)
            # W3 (after m0): remaining weights in first-use order (z then h).
            i_wlast = None
            for lo, hi, dst, src in (
                (0, F, usb, u_r), (0, F, wsb, w_r),
                (2 * F, 3 * F, wsb, w_r), (2 * F, 3 * F, usb, u_r),
            ):
                i_wlast = nc.gpsimd.dma_start(out=dst[:, :, lo:hi], in_=src[:, :, lo:hi])
                add_dep_helper(i_wlast.ins, i_m0.ins, reason="stage DMA waves")

            for c in range(len(CHUNK_W)):
                rows = slice(CHUNK_OFF[c], CHUNK_OFF[c] + CHUNK_W[c])
                last = c == len(CHUNK_W) - 1

                ps_r = gate_mms(F, usb, hd, wsb, xd)
                ps_z = gate_mms(0, usb, hd, wsb, xd)
                tau_r = tau_acts(ps_r, BR, "tau_r")
                tau_z = tau_acts(ps_z, BZ, "tau_z")

                if not last:
                    # chunk-1 loads wait for the last weight DMA so the
                    # startup waves aren't diluted; later chunks are free.
                    gate = i_wlast if c == 0 else None
                    dTn, hTn, i_hn = load_chunk_dh(c + 1, after=gate)
                    xTn, mTn, _ = load_chunk_xm(c + 1, after=gate)
                    tiles_n = (dTn, hTn, xTn, mTn)
                    gtiles_n = exp_chunk(tiles_n)

                # rh' = (tau_r + 1) * hd   (U_h is host-scaled by 0.5)
                rh = wk.tile([128, KT, CHUNK_W[c]], dt.bfloat16, tag="rh")
                nc.vector.scalar_tensor_tensor(
                    rh[:], tau_r[:], 1.0, hd[:], OP.add, OP.mult)

                ps_h = gate_mms(2 * F, wsb, xd, usb, rh)

                hd_c, xd_c = hd, xd
                if not last:
                    hd, xd = preproc_chunk(tiles_n, gtiles_n)
                    tiles = tiles_n

                # h_hat = tanh(.+b_h); h_new = hd + (0.5 tau_z + 0.5)(hh-hd)
                cw = CHUNK_W[c]
                if last:
                    # per-u-tile so the kernel tail pipelines
                    for ut in range(KT):
                        hh = tp.tile([128, cw], dt.bfloat16, tag="hh")
                        if specialized:
                            nc.scalar.activation(hh[:], ps_h[ut][:], AF.Tanh)
                        else:
                            nc.scalar.activation(hh[:], ps_h[ut][:], AF.Tanh,
                                                 bias=csb[:, ut, BH:BH + 1])
                        t6 = tp.tile([128, cw], dt.bfloat16, tag="t6")
                        nc.vector.tensor_sub(out=t6[:], in0=hh[:], in1=hd_c[:, ut, :])
                        t7 = tp.tile([128, cw], dt.bfloat16, tag="t7")
                        nc.vector.scalar_tensor_tensor(
                            t7[:], tau_z[:, ut, :], 1.0, t6[:], OP.add, OP.mult)
                        hn = tp.tile([128, cw], dt.bfloat16, tag="hn")
                        nc.vector.scalar_tensor_tensor(
                            hn[:], t7[:], 0.5, hd_c[:, ut, :], OP.mult, OP.add)
                        nc.sync.dma_start(out=o_r[:, ut, rows], in_=hn[:])
                else:
                    hh = wk.tile([128, KT, cw], dt.bfloat16, tag="hhm")
                    for ut in range(KT):
                        if specialized:
                            nc.scalar.activation(hh[:, ut, :], ps_h[ut][:], AF.Tanh)
                        else:
                            nc.scalar.activation(hh[:, ut, :], ps_h[ut][:], AF.Tanh,
                                                 bias=csb[:, ut, BH:BH + 1])
                    t6 = tp.tile([128, KT, cw], dt.bfloat16, tag="t6m")
                    nc.vector.tensor_sub(out=t6[:], in0=hh[:], in1=hd_c[:])
                    t7 = tp.tile([128, KT, cw], dt.bfloat16, tag="t7m")
                    nc.vector.scalar_tensor_tensor(
                        t7[:], tau_z[:], 1.0, t6[:], OP.add, OP.mult)
                    hn = wk.tile([128, KT, cw], dt.bfloat16, tag="hnm")
                    nc.vector.scalar_tensor_tensor(
                        hn[:], t7[:], 0.5, hd_c[:], OP.mult, OP.add)
                    nc.sync.dma_start(out=o_r[:, :, rows], in_=hn[:])

    nc.compile()
    return nc


def _get_nc(spec_key):
    if spec_key not in _STATE:
        _STATE[spec_key] = _build(spec_key)
    return _STATE[spec_key]


def _tp_cast(a):
    """[B, F] f32 view -> [F, B] contiguous bf16."""
    return np.ascontiguousarray(a.T).astype(BF16)


def kernel(**inputs) -> np.ndarray:
    from concourse import bass_utils

    inp = np.asarray(inputs["inputs"], dtype=np.float32)
    h_prev = np.asarray(inputs["h_prev"], dtype=np.float32)
    gx = np.maximum(np.asarray(inputs["gamma_x_decay"], np.float32), 0.0)
    gh = np.maximum(np.asarray(inputs["gamma_h_decay"], np.float32), 0.0)
    mi = np.asarray(inputs["mean_imputation"], np.float32)
    bz = np.asarray(inputs["b_z"], np.float32)
    br = np.asarray(inputs["b_r"], np.float32)
    bh = np.asarray(inputs["b_h"], np.float32)

    specialized = bool(
        np.all(gx == gx[0]) and np.all(gh == gx[0])
        and not np.any(mi) and not np.any(bz) and not np.any(br) and not np.any(bh)
    )
    spec_key = ("spec", float(-gx[0])) if specialized else "gen"
    nc = _get_nc(spec_key)

    xT = _tp_cast(inp[:, :F])
    mT = _tp_cast(inp[:, F:2 * F])
    dT = _tp_cast(inp[:, 2 * F:])
    hT = _tp_cast(h_prev)

    w = np.concatenate(
        [np.asarray(inputs["W_z"]), np.asarray(inputs["W_r"]), np.asarray(inputs["W_h"])],
        axis=1).astype(FP8 if FP8_W else BF16)
    # fold the sigmoid-via-tanh 0.5 rescale of r into U_h
    u = np.concatenate(
        [np.asarray(inputs["U_z"]), np.asarray(inputs["U_r"]),
         0.5 * np.asarray(inputs["U_h"])],
        axis=1).astype(BF16)

    in_maps = []
    for c in range(N_CORES):
        cols = slice(c * BC, (c + 1) * BC)
        im = {"x": xT[:, cols], "m": mT[:, cols], "d": dT[:, cols], "h": hT[:, cols],
              "w": w, "u": u}
        if not specialized:
            # half-biases for z/r: tanh((pre+b)/2) takes b/2 as the ACT bias
            consts = np.stack([-gx, -gh, mi, 0.5 * bz, 0.5 * br, bh], axis=-1)
            im["c"] = np.ascontiguousarray(
                consts.reshape(KT, 128, 6).transpose(1, 0, 2))
        in_maps.append(im)

    res = bass_utils.run_bass_kernel_spmd(
        nc, in_maps, core_ids=list(range(N_CORES)), **_STATE.get("run_kwargs", {})
    )
    _STATE["last_results"] = res

    out = np.empty((B, F), np.float32)
    for c in range(N_CORES):
        out[c * BC:(c + 1) * BC, :] = res.results[c]["o"].T.astype(np.float32)
    return out
